# revision 17
# baseline (speedup 1.0000x reference)
"""Banded (sparse) multi-head attention block on 8 TRN2 NeuronCores.

Reference computation (B=4, N=1024, C=1024, H=16, D=64, epoch=25 -> band w=8):
    qkv = x @ Wqkv.T                      [B,N,3C], per-head interleaved split
    q,k,v per head; score = q k^T / sqrt(D); band mask |i-j|<=8; softmax
    ctx = attn @ v; out = ctx @ Wproj.T + bproj
Sharding: core = (b, s) owns tokens [s*512, (s+1)*512) of batch b plus an
8-token halo each side; no collectives.

v3 changes vs v2:
  - 112-query attention blocks (4x112 + 64) whose key window is exactly
    128 (112+2*HALO): score/transpose/ctx need no 16-token tail matmuls,
    cutting ~12k PE cycles (free-size costing charges a 16-contraction
    tail matmul the full 128-free price)
  - softmax normalization folded into the PE transpose: rhs is
    diag(1/den) built by one tensor_scalar divide from the identity, so
    the gpsimd/DVE normalize stage disappears
  - v slabs live on the 112 grid (the 16-row overlap is free: matmul
    cost ignores the partition count); 80-token runt via v^T GEMM +
    PE transposes
  - proj PSUM double-buffer deepened to 3 banks
"""

import sys

if "/opt/trn_rl_repo" not in sys.path:
    sys.path.insert(0, "/opt/trn_rl_repo")

import numpy as np

B, N, C, H, D = 4, 1024, 1024, 16, 64
NO = 512          # owned tokens per core
HALO = 8
NL = NO + 2 * HALO    # 528 local tokens
KL = NL           # k/v length (=528; window never overruns)
KT = 8            # contraction tiles (1024 / 128)
SCALE = D ** -0.5

# attention blocks: (q0 local-query start, qn, k0 key-window start, kn)
BLOCKS = [(8, 112, 0, 128), (120, 112, 112, 128), (232, 112, 224, 128),
          (344, 112, 336, 128), (456, 64, 448, 80)]
MOFF = [0, 128, 256, 384, 512]   # mask column offsets; total 592
MW = 592
RUNT0, RUNTN = 448, 80           # runt v tokens [448, 528)

_CACHE = {}


def _build_nc():
    import concourse.bacc as bacc
    import concourse.tile as tile
    from concourse import mybir
    from concourse.masks import make_identity
    from contextlib import ExitStack
    from collections import deque

    f32 = mybir.dt.float32
    bf16 = mybir.dt.bfloat16
    MUL = mybir.AluOpType.mult
    DIV = mybir.AluOpType.divide
    EXP = mybir.ActivationFunctionType.Exp
    IDENT = mybir.ActivationFunctionType.Identity

    nc = bacc.Bacc(None, target_bir_lowering=False)

    xt_e = nc.declare_dram_parameter("xt", [C, NL], bf16, isOutput=False)
    wqkb_e = nc.declare_dram_parameter("wqkb", [128, H * C], bf16, isOutput=False)
    wvt_e = nc.declare_dram_parameter("wvt", [C, C], bf16, isOutput=False)
    wpb_e = nc.declare_dram_parameter("wpb", [128, 8 * C], bf16, isOutput=False)
    bp_e = nc.declare_dram_parameter("bp", [128, 8], f32, isOutput=False)
    mask_e = nc.declare_dram_parameter("mask", [128, MW], bf16, isOutput=False)
    out_e = nc.declare_dram_parameter("outT", [C, NO], bf16, isOutput=True)

    with tile.TileContext(nc) as tc, ExitStack() as ctx:
        const = ctx.enter_context(tc.tile_pool(name="const", bufs=1))
        xts = ctx.enter_context(tc.tile_pool(name="xts", bufs=1))
        wv_pool = ctx.enter_context(tc.tile_pool(name="wvp", bufs=1))
        wqk_pool = ctx.enter_context(tc.tile_pool(name="wqkp", bufs=1))
        wp_pool = ctx.enter_context(tc.tile_pool(name="wpp", bufs=1))
        qk_pool = ctx.enter_context(tc.tile_pool(name="qksb", bufs=1))
        v_pool = ctx.enter_context(tc.tile_pool(name="vsb", bufs=1))
        ctx_pool = ctx.enter_context(tc.tile_pool(name="ctxsb", bufs=1))
        att_pool = ctx.enter_context(tc.tile_pool(name="att", bufs=8))
        out_pool = ctx.enter_context(tc.tile_pool(name="outp", bufs=4))

        # ---- persistent activation slabs -------------------------------
        # q_sb[hp] = [q_{2hp}|q_{2hp+1}]^T; kx2[hp] = [ k_{2hp} | 0 ] ++
        # [ 0 | k_{2hp+1} ] along free (hw faults on base-partition-64
        # operands, so every score matmul contracts the full 128 partitions
        # at base 0; the two zero-padded variants share one tile so each
        # score needs ONE matmul with a [128, 2, kn] strided rhs)
        q_sb, kx2_sb = [], []
        for hp in range(8):
            tq = qk_pool.tile([128, KL], bf16, tag=f"q{hp}")
            q_sb.append(tq)
            tk = qk_pool.tile([128, 2 * KL], bf16, tag=f"kx2{hp}")
            nc.gpsimd.memset(tk[64:128, 0:KL], 0.0)
            nc.gpsimd.memset(tk[0:64, KL:2 * KL], 0.0)
            kx2_sb.append(tk)
        # v slabs on the 112 grid: slab b holds tokens [112b, 112b+128);
        # v_sb[4] rows 0:80 hold the runt tokens [448, 528)
        v_sb, ctxT = [], []
        for j in range(5):
            tv = v_pool.tile([128, C], bf16, tag=f"v{j}")
            v_sb.append(tv)
        for cb in range(8):
            tcx = ctx_pool.tile([128, NO], bf16, tag=f"ctx{cb}")
            ctxT.append(tcx)

        ident = const.tile([128, 128], bf16)
        make_identity(nc, ident[:])

        # ---- DMAs: xt/wv interleaved so phase V can chase the stream ---
        xt_t, wv_t = [], []
        for k in range(KT):
            t = xts.tile([128, NL], bf16, tag=f"xt{k}")
            xt_t.append(t)
            t = wv_pool.tile([128, C], bf16, tag=f"wv{k}")
            wv_t.append(t)
        # k=0 split into halves, interleaved so the first (jb, nch=0)
        # matmuls are gated only by the first two transfers
        nc.sync.dma_start(out=xt_t[0][:, 0:264], in_=xt_e[0:128, 0:264])
        nc.sync.dma_start(out=wv_t[0][:, 0:512], in_=wvt_e[0:128, 0:512])
        nc.sync.dma_start(out=xt_t[0][:, 264:NL], in_=xt_e[0:128, 264:NL])
        nc.sync.dma_start(out=wv_t[0][:, 512:1024], in_=wvt_e[0:128, 512:1024])
        for k in range(1, KT):
            nc.sync.dma_start(out=xt_t[k][:], in_=xt_e[k * 128:(k + 1) * 128, :])
            nc.sync.dma_start(out=wv_t[k][:], in_=wvt_e[k * 128:(k + 1) * 128, :])
        # wqkb grouped 4 slabs per DMA; wpb one DMA (queue dispatch is the
        # scarce resource: ~1.3us per dma_start across SP+HWDGE)
        wqk4_t = []
        for j in range(4):
            t = wqk_pool.tile([128, 4 * C], bf16, tag=f"wqk4_{j}")
            nc.sync.dma_start(out=t[:], in_=wqkb_e[:, 4 * j * C:(4 * j + 4) * C])
            wqk4_t.append(t)
            if j == 0:
                mask_sb = const.tile([128, MW], bf16)
                nc.sync.dma_start(out=mask_sb[:], in_=mask_e[:])
                bp_sb = const.tile([128, 8], f32)
                nc.sync.dma_start(out=bp_sb[:], in_=bp_e[:])

        def wqk_w(g):
            return wqk4_t[g // 4][:, (g % 4) * C:(g % 4 + 1) * C]

        wp_all = wp_pool.tile([128, 8 * C], bf16, tag="wp_all")
        nc.sync.dma_start(out=wp_all[:], in_=wpb_e[:])

        def wp_w(ob):
            return wp_all[:, ob * C:(ob + 1) * C]

        # ---- phase V + QK + attention; PSUM pools are sequenced so no
        # phase ever waits long on a prior pool's release barrier
        if True:
            # ---- phase V: v slabs on the 112 grid, k-outer over 8 banks
            # so PE chases the xt/wv DMA stream ---------------------------
            psvA_cm = tc.tile_pool(name="psvA", bufs=1, space="PSUM")
            psvA = psvA_cm.__enter__()
            psvA_t = []
            for jb in range(4):
                row = []
                for nch in range(2):
                    tp = psvA.tile([128, 512], f32, tag=f"pva{jb}{nch}")
                    row.append(tp)
                psvA_t.append(row)
            # k=0..5 round-robin (chasing the DMA stream), then finish each
            # tile's k=6,7 and copy it out immediately so the PSUM->SBUF
            # copies overlap the remaining matmuls instead of serializing
            # after the whole k-loop
            for k in range(KT - 2):
                # k=0: nch-outer so the first four matmuls need only the
                # first xt/wv half-transfers
                order = [(jb, nch) for nch in range(2) for jb in range(4)] \
                    if k == 0 else [(jb, nch) for jb in range(4) for nch in range(2)]
                for jb, nch in order:
                    nc.tensor.matmul(
                        psvA_t[jb][nch][:],
                        lhsT=xt_t[k][:, jb * 112:jb * 112 + 128],
                        rhs=wv_t[k][:, nch * 512:(nch + 1) * 512],
                        start=(k == 0), stop=False,
                    )
            ci = 0
            for jb in range(4):
                for nch in range(2):
                    for k in (KT - 2, KT - 1):
                        nc.tensor.matmul(
                            psvA_t[jb][nch][:],
                            lhsT=xt_t[k][:, jb * 112:jb * 112 + 128],
                            rhs=wv_t[k][:, nch * 512:(nch + 1) * 512],
                            start=False, stop=(k == KT - 1),
                        )
                    dst = v_sb[jb][:, nch * 512:(nch + 1) * 512]
                    src = psvA_t[jb][nch][:]
                    # GPSIMD cannot touch PSUM: ACT/DVE only
                    if ci % 2 == 0:
                        nc.scalar.copy(out=dst, in_=src)
                    else:
                        nc.vector.tensor_copy(out=dst, in_=src)
                    ci += 1
            psvA_cm.__exit__(None, None, None)

            psqk_cm = tc.tile_pool(name="psqk", bufs=2, space="PSUM")
            psqk = psqk_cm.__enter__()

            def emit_qk(g):
                wt = wqk_w(g)
                # q (even g) only needed for cols 8..520
                chunks = ((8, 256), (264, 256)) if g % 2 == 0 else ((0, 264), (264, 264))
                for n0, nn in chunks:
                    ps = psqk.tile([128, 264], f32, tag="psqk")
                    for k in range(KT):
                        nc.tensor.matmul(
                            ps[:, :nn],
                            lhsT=wt[:, k * 128:(k + 1) * 128],
                            rhs=xt_t[k][:, n0:n0 + nn],
                            start=(k == 0), stop=(k == KT - 1),
                        )
                    if g % 2 == 0:
                        nc.scalar.copy(
                            out=q_sb[g // 2][:, n0:n0 + nn], in_=ps[0:128, :nn])
                    else:
                        nc.scalar.copy(
                            out=kx2_sb[g // 2][0:64, n0:n0 + nn],
                            in_=ps[0:64, :nn])
                        nc.vector.tensor_copy(
                            out=kx2_sb[g // 2][64:128, KL + n0:KL + n0 + nn],
                            in_=ps[64:128, :nn])

            emit_qk(0)

            # ---- runt tokens 448..528 via v^T GEMM + PE transposes -----
            with tc.tile_pool(name="psvR", bufs=1, space="PSUM") as psvR:
                vt_ps = []
                for half in range(2):
                    vt_ps.append(psvR.tile([128, 4 * RUNTN], f32,
                                           name=f"vtp{half}", tag=f"vtp{half}"))
                for cb in range(8):
                    for k in range(KT):
                        nc.tensor.matmul(
                            vt_ps[cb // 4][:, (cb % 4) * RUNTN:(cb % 4 + 1) * RUNTN],
                            lhsT=wv_t[k][:, cb * 128:(cb + 1) * 128],
                            rhs=xt_t[k][:, RUNT0:RUNT0 + RUNTN],
                            start=(k == 0), stop=(k == KT - 1),
                        )
                vt_sb = v_pool.tile([128, 8 * RUNTN], bf16, tag="vt_runt")
                for half in range(2):
                    eng = nc.vector.tensor_copy if half else nc.scalar.copy
                    eng(out=vt_sb[:, half * 4 * RUNTN:(half + 1) * 4 * RUNTN],
                        in_=vt_ps[half][:])
                trp_t = []
                for half in range(2):
                    tp = psvR.tile([RUNTN, 512], bf16, tag=f"trp{half}")
                    trp_t.append(tp)
                for cb in range(8):
                    nc.tensor.transpose(
                        trp_t[cb // 4][:, (cb % 4) * 128:(cb % 4 + 1) * 128],
                        vt_sb[:, cb * RUNTN:(cb + 1) * RUNTN], ident[:])
                for half in range(2):
                    nc.vector.tensor_copy(
                        out=v_sb[4][0:RUNTN, half * 512:(half + 1) * 512],
                        in_=trp_t[half][:])

            emit_qk(1)

            pst_cm = tc.tile_pool(name="pst", bufs=2, space="PSUM")
            pst_pool = pst_cm.__enter__()
            psc_cm = tc.tile_pool(name="psc", bufs=2, space="PSUM")
            psc_pool = psc_cm.__enter__()
            pss_cm = tc.tile_pool(name="pss", bufs=2, space="PSUM")
            pss_pool = pss_cm.__enter__()

            if True:
                state = {}
                pc_cur = [None]

                def emit_score_softmax(it, pool=None):
                    hp, blk = it
                    q0, qn, k0, kn = BLOCKS[blk]
                    off = MOFF[blk]
                    ps = (pool or pss_pool).tile([112, 256], f32,
                                                 tag="psqk" if pool else "ps_s")
                    rhs2 = kx2_sb[hp][:, :].rearrange(
                        "p (two w) -> p two w", two=2)[:, :, k0:k0 + kn]
                    nc.tensor.matmul(
                        ps[0:qn, 0:2 * kn],
                        lhsT=q_sb[hp][:, q0:q0 + qn],
                        rhs=rhs2,
                        start=True, stop=True,
                    )
                    ex = att_pool.tile([112, 256], bf16, tag="ex")
                    nc.scalar.activation(out=ex[0:qn, 0:2 * kn],
                                         in_=ps[0:qn, 0:2 * kn], func=EXP)
                    tmp = att_pool.tile([112, 256], bf16, tag="tmp")
                    den = att_pool.tile([112, 2], f32, tag="den")
                    rec = att_pool.tile([112, 2], f32, tag="rec")
                    d2 = att_pool.tile([112, 224], bf16, tag="d2")
                    for hi in range(2):
                        nc.vector.scalar_tensor_tensor(
                            out=tmp[0:qn, hi * kn:(hi + 1) * kn],
                            in0=ex[0:qn, hi * kn:(hi + 1) * kn],
                            scalar=1.0,
                            in1=mask_sb[0:qn, off:off + kn],
                            op0=MUL, op1=MUL,
                            accum_out=den[0:qn, hi:hi + 1],
                        )
                    nc.vector.reciprocal(out=rec[0:qn, :], in_=den[0:qn, :])
                    for hi in range(2):
                        eng = nc.vector if hi == 0 else nc.gpsimd
                        # diag(1/den): normalization folds into the PE
                        # transpose below (rhs = d2 instead of identity)
                        eng.tensor_scalar_mul(
                            out=d2[0:qn, hi * qn:(hi + 1) * qn],
                            in0=ident[0:qn, 0:qn],
                            scalar1=rec[0:qn, hi:hi + 1],
                        )
                    # normalized transpose: pt = tmp^T scaled per-column
                    pt = pst_pool.tile([128, 224], f32, tag="pt")
                    for hi in range(2):
                        nc.tensor.matmul(
                            pt[0:kn, hi * qn:(hi + 1) * qn],
                            lhsT=tmp[0:qn, hi * kn:(hi + 1) * kn],
                            rhs=d2[0:qn, hi * qn:(hi + 1) * qn],
                            start=True, stop=True,
                        )
                    atT = att_pool.tile([128, 224], bf16, tag="atT")
                    copy_eng = (hp * 5 + blk) % 2
                    eng = nc.scalar.copy if copy_eng == 0 else nc.vector.tensor_copy
                    eng(out=atT[0:kn, 0:2 * qn], in_=pt[0:kn, 0:2 * qn])
                    state[it] = atT

                def emit_transpose_ctx(it, copy_eng=0):
                    hp, blk = it
                    q0, qn, k0, kn = BLOCKS[blk]
                    atT = state.pop(it)
                    if blk == 0:
                        pc = psc_pool.tile([128, 512], f32, tag="pc")
                        pc_cur[0] = pc
                    pc = pc_cur[0]
                    for hi in range(2):
                        h = 2 * hp + hi
                        nc.tensor.matmul(
                            pc[hi * 64:(hi + 1) * 64, q0 - HALO:q0 - HALO + qn],
                            lhsT=v_sb[blk][0:kn, h * 64:(h + 1) * 64],
                            rhs=atT[0:kn, hi * qn:(hi + 1) * qn],
                            start=True, stop=True,
                        )
                    if blk == len(BLOCKS) - 1:
                        eng2 = nc.scalar.copy if hp % 2 else nc.vector.tensor_copy
                        eng2(out=ctxT[hp][:], in_=pc[:])

                pending = deque()
                for hp in range(8):
                    for g in (2 * hp + 2, 2 * hp + 3):
                        if g < H:
                            emit_qk(g)
                    for blk in range(len(BLOCKS)):
                        # the final head-pair's scores draw from the (now
                        # idle) QK pool so the score pool releases before
                        # phase O needs its banks
                        late = hp == 7
                        emit_score_softmax((hp, blk), pool=psqk if late else None)
                        pending.append((hp, blk))
                        if len(pending) > 6:
                            emit_transpose_ctx(pending.popleft())

                # free the score PSUM banks (LIFO), then interleave phase
                # O with the drain of the last head-pair's attention items
                pss_cm.__exit__(None, None, None)

                with tc.tile_pool(name="psg2", bufs=2, space="PSUM") as psg2:
                    po_t = []

                    def emit_proj(ob, ks):
                        ks = list(ks)
                        if ks[0] == 0:
                            ps = psg2.tile([128, NO], f32, tag="po")
                            po_t.append(ps)
                        ps = po_t[ob]
                        for k in ks:
                            nc.tensor.matmul(
                                ps[:],
                                lhsT=wp_w(ob)[:, k * 128:(k + 1) * 128],
                                rhs=ctxT[k][:],
                                start=(k == 0), stop=(k == KT - 1),
                            )
                        if ks[-1] == KT - 1:
                            ot = out_pool.tile([128, NO], bf16, tag="ot")
                            # rotate the final bias-adds across ACT/DVE so
                            # the tail is not serialized on one engine
                            if ob % 2 == 0:
                                nc.scalar.activation(
                                    out=ot[:], in_=ps[:], func=IDENT,
                                    bias=bp_sb[:, ob:ob + 1])
                            else:
                                nc.vector.tensor_scalar_add(
                                    out=ot[:], in0=ps[:],
                                    scalar1=bp_sb[:, ob:ob + 1])
                            nc.sync.dma_start(
                                out=out_e[ob * 128:(ob + 1) * 128, :], in_=ot[:])

                    # pending holds the last 6 items; ctxT[0..5] land
                    # early, ctxT[6..7] trickle in during the drain
                    emit_transpose_ctx(pending.popleft())            # (6,4)
                    emit_proj(0, range(0, 5))
                    emit_transpose_ctx(pending.popleft())            # (7,0)
                    emit_proj(1, range(0, 5))
                    emit_transpose_ctx(pending.popleft())            # (7,1)
                    emit_proj(0, [5, 6])
                    emit_transpose_ctx(pending.popleft())            # (7,2)
                    emit_proj(1, [5, 6])
                    emit_transpose_ctx(pending.popleft())            # (7,3)
                    emit_proj(2, range(0, 5))
                    while pending:                                   # (7,4)
                        emit_transpose_ctx(pending.popleft())
                    emit_proj(0, [KT - 1])
                    emit_proj(1, [KT - 1])
                    emit_proj(2, [5, 6, 7])
                    for ob in range(3, 8):
                        emit_proj(ob, range(KT))

                psc_cm.__exit__(None, None, None)
                pst_cm.__exit__(None, None, None)
            psqk_cm.__exit__(None, None, None)

    nc.compile()
    return nc


def _get_nc():
    if "nc" not in _CACHE:
        _CACHE["nc"] = _build_nc()
    return _CACHE["nc"]


def _band_width(epoch):
    if epoch is None or epoch >= 50:
        return None
    if epoch < 20:
        return 6
    if epoch < 30:
        return 8
    if epoch < 40:
        return 10
    return 12


def _numpy_ref(x, Wqkv, Wproj, bproj, w):
    """Pure-numpy fallback for band widths this kernel wasn't compiled for."""
    b, n, c = x.shape
    d = c // H
    qkv = np.einsum("bnc,oc->bno", x, Wqkv)
    qkv = qkv.reshape(b, n, H, 3 * d).transpose(0, 2, 1, 3)
    q, k, v = np.split(qkv, 3, axis=-1)
    score = np.einsum("bhid,bhjd->bhij", q, k) * (d ** -0.5)
    if w is not None:
        idx = np.arange(n)
        mask = np.abs(idx[:, None] - idx[None, :]) <= w
        score = np.where(mask[None, None], score, np.float32(-1e9))
    score -= score.max(axis=-1, keepdims=True)
    e = np.exp(score)
    attn = e / e.sum(axis=-1, keepdims=True)
    ctxv = np.einsum("bhij,bhjd->bhid", attn, v)
    ctxv = ctxv.transpose(0, 2, 1, 3).reshape(b, n, c)
    return (np.einsum("bnc,oc->bno", ctxv, Wproj) + bproj).astype(np.float32)


def _prep_in_maps(x, Wqkv, Wproj, bproj):
    import ml_dtypes
    bf = ml_dtypes.bfloat16
    x = np.ascontiguousarray(np.asarray(x, dtype=np.float32))
    Wqkv = np.asarray(Wqkv, dtype=np.float32)
    Wproj = np.asarray(Wproj, dtype=np.float32)
    bproj = np.asarray(bproj, dtype=np.float32)

    # qk weight output-blocks g: even g -> [q_{2hp} | q_{2hp+1}] (prescaled),
    # odd g -> [k_{2hp} | k_{2hp+1}]
    wsplit = Wqkv.reshape(H, 3, D, C)
    wq = wsplit[:, 0] * np.float32(SCALE)                      # [H, D, C]
    wk = wsplit[:, 1]                                          # [H, D, C]
    wv = wsplit[:, 2]                                          # [H, D, C]
    wg = np.empty((H, 128, C), dtype=np.float32)
    wg[0::2] = wq.reshape(8, 128, C)
    wg[1::2] = wk.reshape(8, 128, C)
    # wqkb[g, p, k*128+m] = wg[g, m, k*128+p]: per-g contiguous [128, C]
    # slabs whose col-block k is the k-th contraction tile's lhsT
    wqkb = np.ascontiguousarray(
        wg.transpose(0, 2, 1).reshape(H, KT, 128, 128).transpose(0, 2, 1, 3)
        .reshape(H, 128, C).transpose(1, 0, 2).reshape(128, H * C)).astype(bf)
    wvt = np.ascontiguousarray(wv.reshape(H * D, C).T).astype(bf)  # [C, C]
    wpb = np.ascontiguousarray(                                 # Wproj^T blocked
        Wproj.T.reshape(KT, 128, 8, 128).transpose(2, 1, 0, 3)
        .reshape(8, 128, C).transpose(1, 0, 2).reshape(128, 8 * C)).astype(bf)
    bp = np.ascontiguousarray(bproj.reshape(8, 128).T)         # [128, 8]

    # masks per sequence-half s: 1.0 where in-band and the k column is a
    # real token, else 0.0; one [qn, kn] panel per 112-grid block
    masks = []
    for s in (0, 1):
        m = np.zeros((128, MW), dtype=np.float32)
        for (q0, qn, k0, kn), off in zip(BLOCKS, MOFF):
            qi = np.arange(qn)[:, None]
            jj = np.arange(kn)[None, :]
            band = (jj >= qi) & (jj <= qi + 2 * HALO)
            mloc = k0 + jj                                     # local k index
            valid = (mloc >= HALO) if s == 0 else (mloc < NO + HALO)
            m[:qn, off:off + kn] = (band & valid).astype(np.float32)
        masks.append(m.astype(bf))

    in_maps = []
    for core in range(8):
        b, s = core // 2, core % 2
        xloc = np.zeros((NL, C), dtype=np.float32)
        g0 = s * NO - HALO
        lo, hi = max(0, g0), min(N, g0 + NL)
        xloc[lo - g0:hi - g0] = x[b, lo:hi]
        in_maps.append({
            "xt": np.ascontiguousarray(xloc.T).astype(bf),
            "wqkb": wqkb, "wvt": wvt, "wpb": wpb, "bp": bp,
            "mask": masks[s],
        })
    return in_maps


def kernel(x, Wqkv, Wproj, bproj, epoch):
    ep = None if epoch is None else int(np.asarray(epoch))
    w = _band_width(ep)
    if w != HALO:
        return _numpy_ref(np.asarray(x, np.float32), np.asarray(Wqkv, np.float32),
                          np.asarray(Wproj, np.float32),
                          np.asarray(bproj, np.float32), w)

    from concourse.bass_utils import run_bass_kernel_spmd

    nc = _get_nc()
    in_maps = _prep_in_maps(x, Wqkv, Wproj, bproj)
    res = run_bass_kernel_spmd(nc, in_maps, core_ids=list(range(8)))
    _CACHE["last_results"] = res

    out = np.empty((B, N, C), dtype=np.float32)
    for core in range(8):
        b, s = core // 2, core % 2
        out[b, s * NO:(s + 1) * NO, :] = \
            np.asarray(res.results[core]["outT"], dtype=np.float32).T
    return out
